# revision 12
# baseline (speedup 1.0000x reference)
"""Dense dot-product attention with key-length masking on 8 Trainium2 cores.

Problem: q,k,v [16, 2048, 128] fp32, valid_lens [16,1] int32.
  out = softmax(mask(q@k.T/sqrt(d))) @ v   (masked keys -> -1e6 before softmax)

Strategy (data parallel over batch, 2 batches per core):
- Host pre-transposes q,k to [d, seq] so the device never transposes inputs.
- Device computes S^T tiles (keys on partitions) so the mask is a
  per-partition scale/bias folded directly into the exp() activation:
     E = exp(S_raw * scale_k + bias_k),  scale_k = m_k/sqrt(d), bias_k = -30*(1-m_k)
  For valid_len==0 the host sets scale=bias=0 -> E=1 -> uniform softmax,
  matching the reference's where(mask, w, NEG) semantics exactly.
- O^T accumulates over key tiles with V as the stationary operand (f32r,
  full PE rate).  Softmax denominators come from a DVE pairwise tree over
  the E tiles plus a ones-vector matmul; the host does the final divide
  and the output transpose.
- valid_lens-aware specialization: batches sorted by ceil(L/128), the 8
  largest in slot 0, the 8 smallest in slot 1; each slot's key-tile trip
  count is baked at build time (the max over its 8 cores), so on average
  ~30% of the key loop is skipped with a single SPMD program.
"""

import math
import sys
import types

import numpy as np

import concourse.bass as bass
import concourse.mybir as mybir
import concourse.tile as tile
from concourse import bacc
from concourse.bass_utils import run_bass_kernel_spmd

B, Q, K, D = 16, 2048, 2048, 128
NCORES = 8
SLOTS = 2          # batches per core
QCH = 512          # query chunk (moving-operand free dim)
NQCH = Q // QCH
KT = K // 128      # max key tiles
SCALE = 1.0 / math.sqrt(D)
NEG_BIAS = -30.0   # exp(-30) ~ 1e-13: invisible next to real softmax terms

F32 = mybir.dt.float32
F32R = mybir.dt.float32r


def _install_hook_stub():
    """bass_utils' axon trace path imports antenv.axon_hooks, which is not
    shipped in this container.  Provide a no-op stub so an ambient
    BASS_TRACE=1 doesn't crash; test harnesses may overwrite the hook."""
    if "antenv.axon_hooks" in sys.modules:
        return
    mod = types.ModuleType("antenv.axon_hooks")
    _hook = [None]
    mod.set_axon_ntff_profile_hook = lambda h: _hook.__setitem__(0, h)
    mod.get_axon_ntff_profile_hook = lambda: _hook[0]
    sys.modules["antenv.axon_hooks"] = mod


_install_hook_stub()

_build_cache = {}
last_result = None  # BassKernelResults of the most recent run (for harnesses)


def _build(trips):
    """One SPMD program: per slot s, iterate trips[s] key tiles."""
    nc = bacc.Bacc(num_devices=NCORES)

    qT = nc.declare_dram_parameter("qT", [SLOTS, D, Q], F32R, isOutput=False)
    kT = nc.declare_dram_parameter("kT", [SLOTS, D, K], F32R, isOutput=False)
    v = nc.declare_dram_parameter("v", [SLOTS, K, D], F32R, isOutput=False)
    sc = nc.declare_dram_parameter("sc", [SLOTS, 128, KT], F32, isOutput=False)
    bi = nc.declare_dram_parameter("bi", [SLOTS, 128, KT], F32, isOutput=False)
    oT = nc.declare_dram_parameter("oT", [SLOTS, D, Q], F32, isOutput=True)
    # per-chunk partial softmax denominators: host finishes the 128-way sum
    esum = nc.declare_dram_parameter("esum", [SLOTS, NQCH, 128, QCH], F32, isOutput=True)

    with tile.TileContext(nc) as tc:
        with (
            tc.tile_pool(name="consts", bufs=1) as consts,
            tc.tile_pool(name="inputs", bufs=2) as inpool,
            tc.tile_pool(name="epool", bufs=2) as epool,
            tc.tile_pool(name="treep", bufs=2) as treepool,
            tc.tile_pool(name="osb", bufs=2) as opool,
            tc.tile_pool(name="sps", bufs=4, space="PSUM") as pspool,
            tc.tile_pool(name="oacc", bufs=2, space="PSUM") as psacc,
        ):
            for s in range(SLOTS):
                t = trips[s]
                qT_sb = inpool.tile([128, Q], F32R, tag="qT")
                kT_sb = inpool.tile([128, t * 128], F32R, tag="kT")
                v_sb = inpool.tile([128, t, D], F32R, tag="v")
                sc_sb = inpool.tile([128, KT], F32, tag="sc")
                bi_sb = inpool.tile([128, KT], F32, tag="bi")
                nc.sync.dma_start(out=qT_sb[:], in_=qT[s])
                nc.sync.dma_start(out=kT_sb[:], in_=kT[s][:, : t * 128])
                nc.sync.dma_start(
                    out=v_sb[:],
                    in_=v[s].rearrange("(i p) d -> p i d", p=128)[:, :t, :],
                )
                nc.sync.dma_start(out=sc_sb[:], in_=sc[s])
                nc.sync.dma_start(out=bi_sb[:], in_=bi[s])

                for c in range(NQCH):
                    qs = bass.ts(c, QCH)
                    e_sb = epool.tile([128, t, QCH], F32R, tag="e")
                    o_ps = psacc.tile([128, QCH], F32, tag="o")
                    for i in range(t):
                        s_ps = pspool.tile([128, QCH], F32, tag="s")
                        nc.tensor.matmul(
                            s_ps[:],
                            kT_sb[:, bass.ts(i, 128)],
                            qT_sb[:, qs],
                            start=True,
                            stop=True,
                        )
                        nc.scalar.activation(
                            e_sb[:, i, :],
                            s_ps[:],
                            mybir.ActivationFunctionType.Exp,
                            bias=bi_sb[:, i : i + 1],
                            scale=sc_sb[:, i : i + 1],
                        )
                        nc.tensor.matmul(
                            o_ps[:],
                            v_sb[:, i, :],
                            e_sb[:, i, :],
                            start=(i == 0),
                            stop=(i == t - 1),
                        )

                    # denominator: pairwise tree over E tiles down to one
                    # [128, QCH] survivor; host finishes the partition sum.
                    # Writes go to a separate f32 tile: the BIR verifier
                    # forbids non-f32r writers into a matmul-consumed location.
                    aps = [e_sb[:, i, :].bitcast(F32) for i in range(t)]
                    cur = []
                    if t > 1:
                        tr = treepool.tile([128, (t + 1) // 2, QCH], F32, tag="tr")
                        for j in range(t // 2):
                            nc.vector.tensor_add(tr[:, j, :], aps[2 * j], aps[2 * j + 1])
                            cur.append(tr[:, j, :])
                        if t % 2:
                            cur.append(aps[-1])
                        while len(cur) > 1:
                            nxt = []
                            for j in range(len(cur) // 2):
                                nc.vector.tensor_add(cur[2 * j], cur[2 * j], cur[2 * j + 1])
                                nxt.append(cur[2 * j])
                            if len(cur) % 2:
                                nxt.append(cur[-1])
                            cur = nxt
                    else:
                        cur = aps
                    nc.sync.dma_start(out=esum[s, c], in_=cur[0])

                    o_sb = opool.tile([128, QCH], F32, tag="osb")
                    nc.vector.tensor_copy(o_sb[:], o_ps[:])
                    nc.sync.dma_start(out=oT[s][:, qs], in_=o_sb[:])

    nc.compile()
    return nc


def kernel(q, k, v, valid_lens):
    q = np.ascontiguousarray(q, dtype=np.float32)
    k = np.ascontiguousarray(k, dtype=np.float32)
    v = np.ascontiguousarray(v, dtype=np.float32)
    L = np.asarray(valid_lens).reshape(-1).astype(np.int64)

    # per-batch key-tile need; L==0 must cover all keys (uniform softmax)
    need = np.where(L == 0, KT, np.minimum(KT, (L + 127) // 128)).astype(np.int64)
    order = np.argsort(-need, kind="stable")  # descending
    slot_batches = [order[:NCORES], order[NCORES:]]
    trips = tuple(int(need[sb].max()) for sb in slot_batches)

    key = trips
    if key not in _build_cache:
        _build_cache[key] = _build(trips)
    nc = _build_cache[key]

    kidx = np.arange(K)
    in_maps = []
    for c in range(NCORES):
        batches = [int(slot_batches[s][c]) for s in range(SLOTS)]
        qT = np.stack([np.ascontiguousarray(q[b].T) for b in batches])
        kT = np.stack([np.ascontiguousarray(k[b].T) for b in batches])
        vv = np.stack([v[b] for b in batches])
        scs, bis = [], []
        for b in batches:
            lb = int(L[b])
            if lb == 0:
                scv = np.zeros(K, np.float32)
                biv = np.zeros(K, np.float32)
            else:
                m = (kidx < lb).astype(np.float32)
                scv = m * np.float32(SCALE)
                biv = (1.0 - m) * np.float32(NEG_BIAS)
            scs.append(scv.reshape(KT, 128).T)  # [128, KT]
            bis.append(biv.reshape(KT, 128).T)
        in_maps.append(
            {
                "qT": qT,
                "kT": kT,
                "v": vv,
                "sc": np.ascontiguousarray(np.stack(scs), dtype=np.float32),
                "bi": np.ascontiguousarray(np.stack(bis), dtype=np.float32),
            }
        )

    res = run_bass_kernel_spmd(nc, in_maps, list(range(NCORES)))
    global last_result
    last_result = res

    out = np.empty((B, Q, D), np.float32)
    for c in range(NCORES):
        r = res.results[c]
        for s in range(SLOTS):
            b = int(slot_batches[s][c])
            sums = r["esum"][s].sum(axis=1).reshape(-1)  # [NQCH,128,QCH] -> [Q]
            out[b] = (r["oT"][s] / sums[None, :]).T
    return out
